# revision 12
# baseline (speedup 1.0000x reference)
"""Llama RoPE attention (B=2, S=2048, H=2048, 16 heads) on 8 NeuronCores.

Tensor-parallel over heads: core m owns heads {2m, 2m+1}. v5 design:

  dtypes: x/w bf16, q/k/v/e/cos/sin f16, all matmuls 1 cyc/row; PSUM fp32.
  phase 1 (projections+RoPE): per 512-token block, q and k accumulate
    into [128,2,512] PSUM slabs (both heads side by side). The 64 q/k
    matmuls (N=512) are emitted interleaved 1:1 with the 64 v matmuls
    (N=256) so every v LDWEIGHTS hides under a longer q/k matmul.
    RoPE runs as 4 wide DVE ops per slab into resident f16 q/k.
  phase 2 (attention): units of (batch, head, sq-half=1024), split into
    a scores pass and a PV pass. Scores pass: 16x scores k.T@q ->
    [128,2,512] PSUM slab -> exp on ACT (fused 1/sqrt(d)) -> one of 16
    resident f16 e tiles + DVE f16 denominator accumulation. PV pass:
    32 matmuls v.T@e accumulate in two [128,512] PSUM banks; then a
    ones.T@acc matmul partition-reduces+broadcasts the denominator in
    PSUM and per-half approx-reciprocal + mul + DMA finish the unit.
    Decoupling scores from PV keeps the exp round-trip latency out of
    the PE instruction FIFO.
  schedule: batch-0 projections; then pairs [unit scores | batch-1 proj
    | unit PV+norm] so exp always overlaps projection matmuls; then the
    four batch-1 units software-pipelined (scores of unit j+1 overlap
    PV of unit j).

Output is the transposed flattened attention output [256, 4096] per
core; the host stacks core outputs and transposes back.
"""

import math
import os
import sys

for _p in ("/opt/trn_rl_repo", "/root/.axon_site/_ro/trn_rl_repo"):
    if os.path.isdir(_p) and _p not in sys.path:
        sys.path.insert(0, _p)
        break

import numpy as np

import concourse.bass as bass
import concourse.bacc as bacc
import concourse.mybir as mybir
from concourse import bass_isa, tile
from concourse.bass_utils import run_bass_kernel_spmd

N_CORES = 8
HIDDEN = 2048
N_HEAD = 16
HEAD_DIM = 128
B = 2
S = 2048
NTOK = B * S  # 4096
OPC = 256  # output cols per core = 2 heads * 128
KI = HIDDEN // 128  # 16 contraction tiles
NB = NTOK // 512  # 8 n-blocks of 512 tokens
NBLK = 512
SCALE = 1.0 / math.sqrt(HEAD_DIM)
F32 = mybir.dt.float32
F16 = mybir.dt.float16
BF16 = mybir.dt.bfloat16
EXP = mybir.ActivationFunctionType.Exp

_CACHE = {}

# test.py can read this after calling kernel() with BASS_TRACE=1
LAST_RESULT = None


def _build_nc():
    nc = bacc.Bacc("TRN2", target_bir_lowering=False, debug=False,
                   num_devices=N_CORES)
    xT = nc.dram_tensor("xT", [HIDDEN, NTOK], BF16, kind="ExternalInput")
    wqT = nc.dram_tensor("wqT", [HIDDEN, OPC], BF16, kind="ExternalInput")
    wkT = nc.dram_tensor("wkT", [HIDDEN, OPC], BF16, kind="ExternalInput")
    wvT = nc.dram_tensor("wvT", [HIDDEN, OPC], BF16, kind="ExternalInput")
    cosD = nc.dram_tensor("cosD", [HEAD_DIM, 2 * S], F16,
                          kind="ExternalInput")
    sinD = nc.dram_tensor("sinD", [HEAD_DIM, 2 * S], F16,
                          kind="ExternalInput")
    outT = nc.dram_tensor("outT", [OPC, NTOK], F32, kind="ExternalOutput")

    with tile.TileContext(nc) as tc:
        with (
            tc.tile_pool(name="const", bufs=1) as cp,
            tc.tile_pool(name="qk_res", bufs=1) as qkp,
            tc.tile_pool(name="v_res", bufs=1) as vp,
            tc.tile_pool(name="w", bufs=1) as wp,
            tc.tile_pool(name="x", bufs=6) as xp,
            tc.tile_pool(name="rope_tmp", bufs=2) as rtp,
            tc.tile_pool(name="e", bufs=20) as ep,
            tc.tile_pool(name="acc", bufs=2) as accp,
            tc.tile_pool(name="att_tmp", bufs=2) as atp,
            # PSUM budget (8 banks): slabs (q/k proj, scores, denominator)
            # 2 banks x 2 bufs; v-proj pair tiles 1 bank x 2; PV
            # accumulators po0/po1 1 bank each (single-buffered)
            tc.tile_pool(name="ps_big", bufs=2, space="PSUM") as psb,
            tc.tile_pool(name="ps_pv", bufs=2, space="PSUM") as pspv,
            tc.tile_pool(name="ps_po", bufs=1, space="PSUM") as pspo,
        ):
            # resident phase-1 outputs: [128, o(2), 4096] f16
            q_sb = qkp.tile([128, 2, NTOK], F16, tag="q", name="q_sb")
            k_sb = qkp.tile([128, 2, NTOK], F16, tag="k", name="k_sb")
            v_sb = vp.tile([128, 32, 256], F16, tag="v")  # [n%128, jg, 2h*d]

            cos_sb = cp.tile([128, 2, S], F16, tag="cos")
            sin_sb = cp.tile([128, 2, S], F16, tag="sin")
            ones_sb = cp.tile([128, 128], F16, tag="ones")
            nc.vector.memset(ones_sb[:], 1.0)

            def load_w(nm, drt, chunks=1):
                t = wp.tile([128, KI, 256], BF16, tag=f"w{nm}", name=f"w_{nm}")
                kc = KI // chunks
                for ch in range(chunks):
                    nc.sync.dma_start(
                        t[:, ch * kc:(ch + 1) * kc, :],
                        drt[ch * kc * 128:(ch + 1) * kc * 128, :]
                        .rearrange("(t p) o -> p t o", p=128),
                    )
                return t

            def load_x(nb):
                n0 = nb * NBLK
                xc = []
                for c in range(4):
                    xt = xp.tile([128, 4, NBLK], BF16, tag="x", name="xt")
                    nc.sync.dma_start(
                        xt[:],
                        xT[c * 512:(c + 1) * 512, n0:n0 + NBLK]
                        .rearrange("(t p) n -> p t n", p=128),
                    )
                    xc.append(xt)
                return xc

            # chunked wq + first x block first so the PE starts early
            w_sb = {"q": load_w("q", wqT, chunks=4)}
            xc_pre = {0: load_x(0)}
            w_sb["k"] = load_w("k", wkT, chunks=2)
            w_sb["v"] = load_w("v", wvT, chunks=2)
            nc.sync.dma_start(cos_sb[:], cosD[:, :].rearrange(
                "p (t s) -> p t s", t=2))
            nc.sync.dma_start(sin_sb[:], sinD[:, :].rearrange(
                "p (t s) -> p t s", t=2))

            def proj_block(nb):
                """Projections + RoPE + v for one 512-token block.

                The q/k matmul stream (N=512) and the v matmul stream
                (N=256) are interleaved 1:1 so each stream's LDWEIGHTS
                overlaps the other stream's matmul.
                """
                n0 = nb * NBLK
                s0 = (nb % 4) * NBLK  # in-batch position offset
                xc = xc_pre.pop(nb) if nb in xc_pre else load_x(nb)
                if nb + 1 < NB and (nb + 1) not in xc_pre:
                    xc_pre[nb + 1] = load_x(nb + 1)

                qk_mms = []  # 64 closures: q then k, o-major, ki inner
                slabs = {}
                for nm in ("q", "k"):
                    slab = psb.tile([128, 2, NBLK], F32, tag="slab",
                                    name="slab")
                    slabs[nm] = slab
                    for o in range(2):
                        for c in range(4):
                            for t in range(4):
                                i = c * 4 + t
                                qk_mms.append((
                                    slab[:, o, :],
                                    w_sb[nm][:, i, o * 128:o * 128 + 128],
                                    xc[c][:, t, :],
                                    i == 0, i == KI - 1))

                v_mms = []  # 64 closures: two [128,512] pv pair tiles
                pv_tiles = []
                for jp in range(2):
                    pv = pspv.tile([128, 512], F32, tag="pv", name="pv")
                    pv_tiles.append(pv)
                    for jj in range(2):
                        j = jp * 2 + jj
                        for c in range(4):
                            for t in range(4):
                                i = c * 4 + t
                                v_mms.append((
                                    pv[:, jj * 256:(jj + 1) * 256],
                                    xc[c][:, t, j * 128:j * 128 + 128],
                                    w_sb["v"][:, i, :],
                                    i == 0, i == KI - 1))

                def rope(nm):
                    slab, outsb = slabs[nm], q_sb if nm == "q" else k_sb
                    t1 = rtp.tile([128, 2, NBLK], F16, tag="t1", name="t1")
                    t2 = rtp.tile([128, 2, NBLK], F16, tag="t2", name="t2")
                    nc.vector.tensor_mul(
                        t1[:], slab[:], cos_sb[:, :, s0:s0 + NBLK])
                    nc.vector.tensor_mul(
                        t2[0:64, :, :], slab[64:128, :, :],
                        sin_sb[0:64, :, s0:s0 + NBLK])
                    nc.vector.tensor_mul(
                        t2[64:128, :, :], slab[0:64, :, :],
                        sin_sb[64:128, :, s0:s0 + NBLK])
                    nc.vector.tensor_add(
                        outsb[:, :, n0:n0 + NBLK], t1[:], t2[:])

                for m in range(64):
                    out, lhsT, rhs, st, sp = qk_mms[m]
                    nc.tensor.matmul(out, lhsT, rhs, start=st, stop=sp)
                    if m == 31:
                        rope("q")  # q slab complete
                    out, lhsT, rhs, st, sp = v_mms[m]
                    nc.tensor.matmul(out, lhsT, rhs, start=st, stop=sp)
                    if m == 31:
                        jg = nb * 4
                        nc.scalar.copy(
                            v_sb[:, jg:jg + 2, :]
                            .rearrange("p a b -> p (a b)"),
                            pv_tiles[0][:])
                rope("k")
                jg = nb * 4 + 2
                nc.scalar.copy(
                    v_sb[:, jg:jg + 2, :].rearrange("p a b -> p (a b)"),
                    pv_tiles[1][:])

            def attn_unit(b, h, half):
                """Fused attention unit (used in proj-interleaved pairs).

                PE emission is software-pipelined one sk deep: scores(sk+1)
                issues before PV(sk).
                """
                sq0 = b * 2048 + half * 1024
                po = [pspo.tile([128, NBLK], F32, tag=f"po{q}",
                                name=f"po{q}") for q in range(2)]
                acc = accp.tile([128, 2 * NBLK], F16, tag="acc", name="acc")

                def scores_exp(sk):
                    kt = k_sb[:, h, b * 2048 + sk * 128:
                              b * 2048 + sk * 128 + 128]
                    ps = psb.tile([128, 2, NBLK], F32, tag="slab",
                                  name="ps_s")
                    for q in range(2):
                        nc.tensor.matmul(
                            ps[:, q, :],
                            kt,
                            q_sb[:, h, sq0 + q * NBLK:sq0 + (q + 1) * NBLK],
                            start=True, stop=True,
                        )
                    e = ep.tile([128, 2 * NBLK], F16, tag="e", name="e")
                    nc.scalar.activation(
                        e[:], ps[:].rearrange("p a b -> p (a b)"), EXP,
                        scale=SCALE)
                    return e

                e_cur = scores_exp(0)
                for sk in range(16):
                    e_next = scores_exp(sk + 1) if sk < 15 else None
                    if sk == 0:
                        nc.vector.tensor_copy(acc[:], e_cur[:])
                    else:
                        nc.vector.tensor_add(acc[:], acc[:], e_cur[:])
                    jg = b * 16 + sk
                    vt = v_sb[:, jg, h * 128:h * 128 + 128]
                    for q in range(2):
                        nc.tensor.matmul(
                            po[q][:],
                            vt,
                            e_cur[:, q * NBLK:(q + 1) * NBLK],
                            start=(sk == 0), stop=(sk == 15),
                        )
                    e_cur = e_next
                _norm_out(b, h, half, acc, po)

            def _norm_out(b, h, half, acc, po):
                """ones-matmul denominator + per-half recip/mul/DMA."""
                sq0 = b * 2048 + half * 1024
                den = psb.tile([128, 2, NBLK], F32, tag="slab", name="den")
                for q in range(2):
                    nc.tensor.matmul(den[:, q, :], ones_sb[:],
                                     acc[:, q * NBLK:(q + 1) * NBLK],
                                     start=True, stop=True)
                for q in range(2):
                    rc = atp.tile([128, NBLK], F32, tag="rc", name="rc")
                    nc.vector.reciprocal_approx_fast(rc[:], den[:, q, :])
                    osb = atp.tile([128, NBLK], F32, tag="osb", name="osb")
                    nc.vector.tensor_mul(osb[:], po[q][:], rc[:])
                    nc.sync.dma_start(
                        outT[h * 128:(h + 1) * 128,
                             sq0 + q * NBLK:sq0 + (q + 1) * NBLK],
                        osb[:])

            # ---- decoupled tail units: scores pass feeds resident e
            # tiles; PV pass runs as a stall-free burst overlapped with
            # the next unit's scores (exp stream stays continuous) ----
            def tail_scores(st, sk_lo, sk_hi):
                b, h, half, acc, es = st
                sq0 = b * 2048 + half * 1024
                for sk in range(sk_lo, sk_hi):
                    kt = k_sb[:, h, b * 2048 + sk * 128:
                              b * 2048 + sk * 128 + 128]
                    ps = psb.tile([128, 2, NBLK], F32, tag="slab",
                                  name="ps_s")
                    for q in range(2):
                        nc.tensor.matmul(
                            ps[:, q, :],
                            kt,
                            q_sb[:, h, sq0 + q * NBLK:sq0 + (q + 1) * NBLK],
                            start=True, stop=True,
                        )
                    e = ep.tile([128, 2 * NBLK], F16, tag="e", name="e")
                    nc.scalar.activation(
                        e[:], ps[:].rearrange("p a b -> p (a b)"), EXP,
                        scale=SCALE)
                    if sk == 0:
                        nc.vector.tensor_copy(acc[:], e[:])
                    else:
                        nc.vector.tensor_add(acc[:], acc[:], e[:])
                    es.append(e)

            def tail_state(b, h, half):
                acc = accp.tile([128, 2 * NBLK], F16, tag="acc", name="acc")
                return (b, h, half, acc, [])

            def tail_pv(st):
                b, h, half, acc, es = st
                po = [pspo.tile([128, NBLK], F32, tag=f"po{q}",
                                name=f"po{q}") for q in range(2)]
                for sk in range(16):
                    jg = b * 16 + sk
                    vt = v_sb[:, jg, h * 128:h * 128 + 128]
                    for q in range(2):
                        nc.tensor.matmul(
                            po[q][:],
                            vt,
                            es[sk][:, q * NBLK:(q + 1) * NBLK],
                            start=(sk == 0), stop=(sk == 15),
                        )
                _norm_out(b, h, half, acc, po)

            # schedule: batch-0 projections; pairs [proj | fused unit];
            # batch-1 units decoupled with 4-sk lookahead
            units = [(b, h, half) for b in range(2) for h in range(2)
                     for half in range(2)]
            for nb in range(4):
                proj_block(nb)
            for i in range(4):
                proj_block(4 + i)
                attn_unit(*units[i])
            sts = [tail_state(*units[i]) for i in range(4, 8)]
            tail_scores(sts[0], 0, 16)
            for j in range(3):
                # zip: one scores step of unit j+1 with one PV step of
                # unit j per period, so ACT's exp stream never runs dry
                bp, hp, halfp, accp_, esp = sts[j]
                po = [pspo.tile([128, NBLK], F32, tag=f"po{q}",
                                name=f"po{q}") for q in range(2)]
                for sk in range(16):
                    tail_scores(sts[j + 1], sk, sk + 1)
                    jg = bp * 16 + sk
                    vt = v_sb[:, jg, hp * 128:hp * 128 + 128]
                    for q in range(2):
                        nc.tensor.matmul(
                            po[q][:],
                            vt,
                            esp[sk][:, q * NBLK:(q + 1) * NBLK],
                            start=(sk == 0), stop=(sk == 15),
                        )
                _norm_out(bp, hp, halfp, accp_, po)
            tail_pv(sts[3])

    nc.compile()
    return nc


def _get_nc():
    if "nc" not in _CACHE:
        _CACHE["nc"] = _build_nc()
    return _CACHE["nc"]


def _np_bf16(a):
    """fp32 -> bf16 (round-to-nearest-even) as uint16-backed array."""
    import ml_dtypes
    return np.asarray(a, dtype=np.float32).astype(ml_dtypes.bfloat16)


def _cos_sin():
    if "cs" not in _CACHE:
        half = np.arange(0, HEAD_DIM, 2, dtype=np.float32)[: HEAD_DIM // 2]
        freq = (1.0 / 10000.0 ** (half / HEAD_DIM)).astype(np.float32)
        t = np.arange(S, dtype=np.float32)
        freqs = np.outer(t, freq).astype(np.float32)  # [S, 64]
        emb = np.concatenate([freqs, freqs], axis=1)  # [S, 128]
        cosT = np.ascontiguousarray(np.cos(emb).astype(np.float32).T)
        sinT = np.ascontiguousarray(np.sin(emb).astype(np.float32).T)
        sinS = np.concatenate([-sinT[0:64], sinT[64:128]], axis=0)
        cosDup = np.concatenate([cosT, cosT], axis=1)  # [128, 2S]
        sinDup = np.concatenate([sinS, sinS], axis=1)
        _CACHE["cs"] = (cosDup.astype(np.float16), sinDup.astype(np.float16))
    return _CACHE["cs"]


def kernel(x, wq, wk, wv):
    global LAST_RESULT
    nc = _get_nc()
    cosDup, sinDup = _cos_sin()
    x2 = _np_bf16(np.ascontiguousarray(
        np.asarray(x, dtype=np.float32).reshape(NTOK, HIDDEN).T))
    in_maps = []
    for m in range(N_CORES):
        sl = slice(m * OPC, (m + 1) * OPC)
        in_maps.append({
            "xT": x2,
            "wqT": _np_bf16(np.ascontiguousarray(np.asarray(wq)[sl].T)),
            "wkT": _np_bf16(np.ascontiguousarray(np.asarray(wk)[sl].T)),
            "wvT": _np_bf16(np.ascontiguousarray(np.asarray(wv)[sl].T)),
            "cosD": cosDup,
            "sinD": sinDup,
        })
    res = run_bass_kernel_spmd(nc, in_maps, core_ids=list(range(N_CORES)))
    LAST_RESULT = res
    big = np.concatenate([r["outT"] for r in res.results], axis=0)
    return np.ascontiguousarray(big.T).reshape(B, S, HIDDEN).astype(np.float32)


if __name__ == "__main__":
    _get_nc()
    print("build OK")
